# revision 2
# baseline (speedup 1.0000x reference)
"""DTCWT 3-level inverse on 8 Trainium2 NeuronCores — v2.

Restructure vs v1: c2q band combos (x1=w1r+w2r, ...) are computed by the
vector engines into row-e/o-blocked, col-interleaved tiles, so every
filtering stage is a plain banded matmul with natural-order matrices —
no e|o column packing, no polyphase recombination, no SBUF shift DMAs.
PE row streams drop from ~17.7k to ~8.8k rows/image.

Precision: z/y chain fp32r (exact fp32 storage, tf32-ish matmul); band
path bf16 (inputs shipped bf16 → half DMA; one bf16 matmul per band).
Mixed-dtype accumulation groups merge the z and band contributions in
PSUM (fp32).

Host side does pure layout permutation (transpose/reorder/dup/cast):
 - yl   -> [64, 16*64] f32 (rows on partitions, images along free)
 - yh0  -> b1 [16, 128, 1536] bf16: [R1|R2|I1|I2], R1=[o0R|o2R|o1R]
 - yh1  -> b2 [16, 128, 768] bf16: same layout at 64 rows, row-duplicated
 - yh2  -> b3 [16, 128, 384] bf16: [o0R o2R|o5R o3R|o0I o2I|o5I o3I|
           o1R o4R|o1I o4I] at 32 rows, 4x row-duplicated
 - out  <- [16, 128, 512] f32 (rows r and r+128 side by side)

Sharding: pure data parallel over batch N (8 cores x 16 channels each).
"""
import sys

for _p in ('/opt/trn_rl_repo',):
    if _p not in sys.path:
        sys.path.append(_p)

import numpy as np
import ml_dtypes
import concourse.bass as bass
import concourse.mybir as mybir
from concourse.tile import TileContext
from concourse.bass_utils import run_bass_kernel_spmd

SQRT_HALF = 0.7071067811865476
N_CORES = 8
IMGS = 16
F32 = mybir.dt.float32
F32R = mybir.dt.float32r
BF16 = mybir.dt.bfloat16
BF = ml_dtypes.bfloat16

MULT = None
ADD = None


# ---------------------------------------------------------------------------
# Host-side matrix construction (float64)
# ---------------------------------------------------------------------------
def _conv_rows_valid(x, h):
    hr = h[::-1]
    taps = h.shape[0]
    n = x.shape[-2] - taps + 1
    out = hr[0] * x[..., 0:n, :]
    for k in range(1, taps):
        out = out + hr[k] * x[..., k:k + n, :]
    return out


def _pad_rows_symmetric(x, m):
    pad = [(0, 0)] * (x.ndim - 2) + [(m, m), (0, 0)]
    return np.pad(x, pad, mode='symmetric')


def _colfilter(x, h):
    return _conv_rows_valid(_pad_rows_symmetric(x, h.shape[0] // 2), h)


def _colifilt(x, ha, hb, highpass):
    m = ha.shape[0]
    m2 = m // 2
    r = x.shape[-2]
    xp = _pad_rows_symmetric(x, m2)
    xe = xp[..., 1:r + m - 2:2, :]
    xo = xp[..., 2:r + m - 1:2, :]
    xa, xb = (xe, xo) if highpass else (xo, xe)
    hao, hae = ha[0::2], ha[1::2]
    hbo, hbe = hb[0::2], hb[1::2]
    y0 = _conv_rows_valid(xb, hao)
    y1 = _conv_rows_valid(xa, hbo)
    y2 = _conv_rows_valid(xb, hae)
    y3 = _conv_rows_valid(xa, hbe)
    y = np.stack([y0, y1, y2, y3], axis=-2)
    return y.reshape(y.shape[:-3] + (2 * r, y.shape[-1]))


def _op_matrix(op, n):
    """M[h_in, h_out] with out = M.T @ x."""
    return np.ascontiguousarray(op(np.eye(n, dtype=np.float64)).T)


# packed matrix tiles: name -> (col_offset, K, N)
# w enumeration in the transposed domain is parity-ordered ([evens|odds]),
# matching the blocked-contiguous combo tiles; row-stage matrices carry the
# matching row parity permutation (and R3* also column parity, so z2's
# columns are parity-ordered for L2).
MATZ_OFF = {'C3z': (0, 64, 128), 'R3lo': (128, 64, 128),
            'R3hi': (256, 64, 128),
            'C2z': (384, 128, 256), 'R2lo': (640, 128, 256),
            'R2hi': (896, 128, 256),
            'Alo_r0': (1152, 128, 256), 'Alo_r1': (1408, 128, 256),
            'Alo_e': (1664, 128, 256), 'Alo_o': (1920, 128, 256),
            'Ahi_e': (2176, 128, 256), 'Ahi_o': (2432, 128, 256)}
MATZ_W = 2688
MATB_OFF = {'C3b': (0, 64, 128), 'C3qlo': (128, 64, 128),
            'C2b': (256, 128, 256), 'C2blo': (512, 128, 256),
            'sAhi_e': (768, 128, 256), 'sAhi_o': (1024, 128, 256),
            'sAlo_e': (1280, 128, 256), 'sAlo_o': (1536, 128, 256)}
MATB_W = 1792
# cols < SPLIT load first (everything level-3) so image 0 starts early
MAT_SPLIT = 384


def build_matrices(g0o, g1o, g0a, g0b, g1a, g1b):
    g0o = np.asarray(g0o, np.float64)
    g1o = np.asarray(g1o, np.float64)
    g0a = np.asarray(g0a, np.float64)
    g0b = np.asarray(g0b, np.float64)
    g1a = np.asarray(g1a, np.float64)
    g1b = np.asarray(g1b, np.float64)
    s = SQRT_HALF
    vs = np.vstack
    Mlo3 = _op_matrix(lambda x: _colifilt(x, g0b, g0a, False), 64)
    Mhi3 = _op_matrix(lambda x: _colifilt(x, g1b, g1a, True), 64)
    Mlo2 = _op_matrix(lambda x: _colifilt(x, g0b, g0a, False), 128)
    Mhi2 = _op_matrix(lambda x: _colifilt(x, g1b, g1a, True), 128)
    Alo = _op_matrix(lambda x: _colfilter(x, g0o), 256)
    Ahi = _op_matrix(lambda x: _colfilter(x, g1o), 256)
    def pe(M):   # parity row permutation
        return np.vstack([M[0::2], M[1::2]])

    def pcol(M):  # parity column permutation
        return np.hstack([M[:, 0::2], M[:, 1::2]])

    mz = {'C3z': Mlo3, 'R3lo': pcol(pe(Mlo3)), 'R3hi': pcol(pe(Mhi3)),
          'C2z': Mlo2, 'R2lo': pe(Mlo2), 'R2hi': pe(Mhi2),
          'Alo_r0': Alo[0:128], 'Alo_r1': Alo[128:256],
          'Alo_e': Alo[0::2], 'Alo_o': Alo[1::2],
          'Ahi_e': Ahi[0::2], 'Ahi_o': Ahi[1::2]}
    mb = {'C3b': vs([s * Mhi3[0::2], s * Mhi3[1::2]]),
          'C3qlo': vs([s * Mlo3[0::2], s * Mlo3[1::2]]),
          'C2b': vs([s * Mhi2[0::2], s * Mhi2[1::2]]),
          'C2blo': vs([s * Mlo2[0::2], s * Mlo2[1::2]]),
          'sAhi_e': s * Ahi[0::2], 'sAhi_o': s * Ahi[1::2],
          'sAlo_e': s * Alo[0::2], 'sAlo_o': s * Alo[1::2]}
    matz = np.zeros((128, MATZ_W), np.float32)
    for k, (off, K, N) in MATZ_OFF.items():
        matz[0:K, off:off + N] = mz[k]
    matb = np.zeros((128, MATB_W), np.float32)
    for k, (off, K, N) in MATB_OFF.items():
        matb[0:K, off:off + N] = mb[k]
    return matz.astype(BF), matb.astype(BF)


# ---------------------------------------------------------------------------
# Host-side input layout permutation (no arithmetic)
# ---------------------------------------------------------------------------
def _band_layout(yh, first=(0, 2, 1), second=(5, 3, 4)):
    """yh [I, 6, r, c, 2] -> [I, r, 4*3*c]: [R1|R2|I1|I2], R1=[oA|oB|oC]."""
    I, _, r, c, _ = yh.shape
    parts = []
    for ri in (0, 1):
        for sel in (first, second):
            t = yh[:, sel][..., ri]               # [I, 3, r, c]
            parts.append(np.transpose(t, (0, 2, 1, 3)).reshape(I, r, 3 * c))
    return np.concatenate(parts, axis=2)


def prep_core(yl, yh0, yh1, yh2):
    """Per-core host layout prep -> device input map (minus matrices)."""
    I = yl.shape[0]
    ylp = np.concatenate([yl[:, :, 0::2], yl[:, :, 1::2]], axis=2)
    yl_t = np.ascontiguousarray(
        np.transpose(ylp, (1, 0, 2)).reshape(64, I * 64).astype(BF))
    def dup_swapped(bl):
        # duplicate rows with quarter-blocks reordered [I2|I1|R1|R2] so the
        # fused x3x4 op is a single (q1 - q0) tensor_sub
        q = bl.shape[2] // 4
        bo = np.concatenate([bl[:, :, 3 * q:], bl[:, :, 2 * q:3 * q],
                             bl[:, :, 0:q], bl[:, :, q:2 * q]], axis=2)
        return np.ascontiguousarray(np.concatenate([bl, bo], axis=1))

    b1 = np.ascontiguousarray(_band_layout(yh0).astype(BF))       # [I,128,1536]
    b2 = dup_swapped(_band_layout(yh1).astype(BF))                # [I,128,768]
    b3 = dup_swapped(_band_layout(yh2).astype(BF))                # [I,64,384]
    return {'yl_t': yl_t, 'b1': b1, 'b2': b2, 'b3': b3}


# ---------------------------------------------------------------------------
# Bass kernel
# ---------------------------------------------------------------------------
def split_excess_waits(nc, max_waits=1):
    """walrus CTRL codegen allows only one sem wait per instruction; move
    excess waits onto NoOps inserted just before the offending instruction."""
    ctr = 0
    for fn in nc.m.functions:
        for bb in fn.blocks:
            new_list = []
            for inst in bb.instructions:
                si = inst.sync_info
                if si is not None and si.on_wait and len(si.on_wait) > max_waits:
                    waits = list(si.on_wait)
                    keep, extra = waits[:max_waits], waits[max_waits:]
                    for i in range(0, len(extra), max_waits):
                        nop = mybir.InstNoOp(
                            name=f"wait_split_{ctr}", ins=[], outs=[])
                        ctr += 1
                        nop.engine = inst.engine
                        nop.sync_info = mybir.SyncInfo(
                            on_wait=extra[i:i + max_waits], on_update=[])
                        nc.register_instruction(nop)
                        new_list.append(nop)
                    inst.sync_info = mybir.SyncInfo(
                        on_wait=keep,
                        on_update=list(si.on_update) if si.on_update else [])
                new_list.append(inst)
            bb.instructions[:] = new_list
    return ctr


def build_nc():
    global MULT, ADD
    MULT = mybir.AluOpType.mult
    ADD = mybir.AluOpType.add

    nc = bass.Bass()
    yl_d = nc.dram_tensor("yl_t", [64, IMGS * 64], BF16, kind="ExternalInput")
    b1_d = nc.dram_tensor("b1", [IMGS, 128, 1536], BF16, kind="ExternalInput")
    b2_d = nc.dram_tensor("b2", [IMGS, 128, 768], BF16, kind="ExternalInput")
    b3_d = nc.dram_tensor("b3", [IMGS, 64, 384], BF16, kind="ExternalInput")
    matz_d = nc.dram_tensor("matz", [128, MATZ_W], BF16, kind="ExternalInput")
    matb_d = nc.dram_tensor("matb", [128, MATB_W], BF16, kind="ExternalInput")
    out_d = nc.dram_tensor("out", [IMGS, 128, 512], F32, kind="ExternalOutput")

    with TileContext(nc) as tc:
        with tc.tile_pool(name="mats", bufs=1) as matpool, \
             tc.tile_pool(name="ylp", bufs=1) as ylpool, \
             tc.tile_pool(name="inp", bufs=IMGS) as inpool, \
             tc.tile_pool(name="zp", bufs=IMGS) as zpool, \
             tc.tile_pool(name="bxp", bufs=3) as bxpool, \
             tc.tile_pool(name="mid", bufs=2) as midpool, \
             tc.tile_pool(name="outp", bufs=2) as outpool, \
             tc.tile_pool(name="pl3", bufs=2, space="PSUM") as pl3pool, \
             tc.tile_pool(name="pl2", bufs=1, space="PSUM") as pl2pool, \
             tc.tile_pool(name="pl1c", bufs=1, space="PSUM") as pl1cpool, \
             tc.tile_pool(name="pl1r", bufs=2, space="PSUM") as pl1rpool:

            # matrices: level-3 columns first so image 0 can start early;
            # bulk (L2/L1) matrices go on the otherwise-idle gpsimd queue
            matz = matpool.tile([128, MATZ_W], BF16, tag="matz")
            matb = matpool.tile([128, MATB_W], BF16, tag="matb")
            nc.sync.dma_start(out=matz[:, 0:MAT_SPLIT],
                              in_=matz_d[:, 0:MAT_SPLIT])
            nc.sync.dma_start(out=matb[:, 0:MAT_SPLIT],
                              in_=matb_d[:, 0:MAT_SPLIT])
            yl_t = ylpool.tile([64, IMGS * 64], BF16, tag="yl")
            nc.sync.dma_start(out=yl_t[:], in_=yl_d[:])

            def rz(name):
                off, K, N = MATZ_OFF[name]
                return matz[0:K, off:off + N]

            def rb(name):
                off, K, N = MATB_OFF[name]
                return matb[0:K, off:off + N]

            def mm(out_ap, lhsT, rhs, start, stop):
                nc.tensor.matmul(out_ap, lhsT, rhs, start=start, stop=stop)

            def stt(eng, out, in0, in1, sub=False):
                # sub: out = in1 - in0 ; else out = in0 + in1
                if sub:
                    eng.tensor_sub(out, in1, in0)
                else:
                    eng.tensor_add(out, in0, in1)

            # Software-pipelined emission: iteration i emits L3(i),
            # L2(i-1), L1(i-2) so each in-order engine stream interleaves
            # stages of different images (tensor never starves behind a
            # combo-bound stage) and the tail drains only L1 of the last
            # two images. Inputs and z tiles stay resident (bufs=IMGS).
            z2s, z1s, b2s, b1s = [], [], [], []

            def emit_l3(img):
                b3 = inpool.tile([64, 384], BF16, tag="b3")
                nc.sync.dma_start(out=b3[:], in_=b3_d[img])
                # bx3 [64,192] = [lh | hl | hh] blocks of [x1(32)|x2(32)],
                # E rows at parts 0:32, O rows at 32:64 (b3 row-duplicated)
                bx3 = bxpool.tile([64, 192], BF16, tag="bx3")
                bx3v = bx3[:].rearrange("p (b t c) -> p b t c", b=3, t=2)
                b3q = b3[:].rearrange("p (h q b c) -> p q b h c",
                                      h=2, q=2, b=3)
                e3, o3 = slice(0, 32), slice(32, 64)
                stt(nc.gpsimd, bx3v[e3], b3q[e3, 1], b3q[e3, 0])
                # o-rows shipped as [I2|I1|R1|R2]: x3x4 = q1 - q0 in one op
                stt(nc.gpsimd, bx3v[o3], b3q[o3, 0], b3q[o3, 1], sub=True)

                p3 = pl3pool.tile([128, 512], F32, tag="p3")
                py13 = p3[0:64, 0:128]
                py23 = p3[0:64, 128:256]
                pz2 = p3[0:128, 256:384]
                yl_s = yl_t[:, img * 64:(img + 1) * 64]
                mm(py13, yl_s, rz('C3z'), True, False)
                mm(py13, bx3[:, 0:64], rb('C3b'), False, True)
                mm(py23, bx3[:, 64:128], rb('C3qlo'), True, False)
                mm(py23, bx3[:, 128:192], rb('C3b'), False, True)
                y133 = midpool.tile([64, 256], BF16, tag="y133")
                nc.vector.tensor_copy(out=y133[:], in_=p3[0:64, 0:256])
                mm(pz2, y133[:, 0:128], rz('R3lo'), True, False)
                mm(pz2, y133[:, 128:256], rz('R3hi'), False, True)
                z2 = zpool.tile([128, 128], BF16, tag="z2")
                nc.scalar.copy(z2[:], pz2)
                z2s.append(z2)
                b2 = inpool.tile([128, 768], BF16, tag="b2")
                nc.sync.dma_start(out=b2[:], in_=b2_d[img])
                b2s.append(b2)
                b1 = inpool.tile([128, 1536], BF16, tag="b1")
                nc.sync.dma_start(out=b1[:], in_=b1_d[img])
                b1s.append(b1)

            def emit_l2(img):
                z2, b2 = z2s[img], b2s[img]
                bx2 = bxpool.tile([128, 384], BF16, tag="bx2")
                bx2v = bx2[:].rearrange("p (b t c) -> p b t c", b=3, t=2)
                b2q = b2[:].rearrange("p (h q b c) -> p q b h c",
                                      h=2, q=2, b=3)
                e, o = slice(0, 64), slice(64, 128)
                stt(nc.vector, bx2v[e], b2q[e, 1], b2q[e, 0])
                stt(nc.gpsimd, bx2v[o], b2q[o, 0], b2q[o, 1], sub=True)

                p2 = pl2pool.tile([128, 1024], F32, tag="p2")
                py12 = p2[:, 0:256]
                py22 = p2[:, 256:512]
                mm(py12, z2[:], rz('C2z'), True, False)
                mm(py12, bx2[:, 0:128], rb('C2b'), False, True)
                mm(py22, bx2[:, 128:256], rb('C2blo'), True, False)
                mm(py22, bx2[:, 256:384], rb('C2b'), False, True)
                y122 = midpool.tile([128, 512], BF16, tag="y122")
                nc.scalar.copy(y122[:], p2[:, 0:512])
                z1 = zpool.tile([128, 512], BF16, tag="z1")  # [z1a | z1b]
                for m in (0, 1):
                    pz1 = p2[:, 512 + m * 256:768 + m * 256]
                    mm(pz1, y122[:, m * 128:(m + 1) * 128], rz('R2lo'),
                       True, False)
                    mm(pz1, y122[:, 256 + m * 128:384 + m * 128], rz('R2hi'),
                       False, True)
                nc.vector.tensor_copy(out=z1[:], in_=p2[:, 512:1024])
                z1s.append(z1)

            def emit_l1(img):
                z1, b1 = z1s[img], b1s[img]
                bx1 = bxpool.tile([128, 1536], BF16, tag="bx1")
                bx1v = bx1[:].rearrange("p (b eo t c) -> p eo b t c",
                                        b=3, eo=2, t=2)
                b1q = b1[:].rearrange("p (h q b c) -> p q b h c",
                                      h=2, q=2, b=3)
                stt(nc.vector, bx1v[:, 0], b1q[:, 1], b1q[:, 0])
                stt(nc.vector, bx1v[:, 1, :, 0, :], b1q[:, 1, :, 1, :],
                    b1q[:, 0, :, 1, :], sub=True)
                stt(nc.gpsimd, bx1v[:, 1, :, 1, :], b1q[:, 0, :, 0, :],
                    b1q[:, 1, :, 0, :], sub=True)
                # (L1 x3/x4 stay separate: b1 rows are not duplicated)

                p1 = pl1cpool.tile([128, 1024], F32, tag="p1")
                y11 = midpool.tile([128, 512], BF16, tag="y11")  # w par-blocks
                y21 = midpool.tile([128, 512], BF16, tag="y21")
                for m in (0, 1):
                    py1 = p1[:, m * 256:(m + 1) * 256]
                    mm(py1, z1[:, m:256:2], rz('Alo_r0'), True, False)
                    mm(py1, z1[:, 256 + m:512:2], rz('Alo_r1'), False, False)
                    mm(py1, bx1[:, 0 + m * 128:128 + m * 128], rb('sAhi_e'),
                       False, False)
                    mm(py1, bx1[:, 256 + m * 128:384 + m * 128], rb('sAhi_o'),
                       False, True)
                    py2 = p1[:, 512 + m * 256:768 + m * 256]
                    mm(py2, bx1[:, 512 + m * 128:640 + m * 128], rb('sAlo_e'),
                       True, False)
                    mm(py2, bx1[:, 768 + m * 128:896 + m * 128], rb('sAlo_o'),
                       False, False)
                    mm(py2, bx1[:, 1024 + m * 128:1152 + m * 128],
                       rb('sAhi_e'), False, False)
                    mm(py2, bx1[:, 1280 + m * 128:1408 + m * 128],
                       rb('sAhi_o'), False, True)
                nc.vector.tensor_copy(out=y11[:], in_=p1[:, 0:512])
                nc.scalar.copy(y21[:], p1[:, 512:1024])

                outb = outpool.tile([128, 512], F32, tag="outb")
                pr = pl1rpool.tile([128, 512], F32, tag="pr")
                for m in (0, 1):
                    po = pr[:, m * 256:(m + 1) * 256]
                    msl = slice(m * 128, (m + 1) * 128)
                    mm(po, y11[:, msl], rz('Alo_e'), True, False)
                    mm(po, y11[:, 256 + m * 128:384 + m * 128], rz('Alo_o'),
                       False, False)
                    mm(po, y21[:, msl], rz('Ahi_e'), False, False)
                    mm(po, y21[:, 256 + m * 128:384 + m * 128], rz('Ahi_o'),
                       False, True)
                nc.vector.tensor_copy(out=outb[:, 0:256], in_=pr[:, 0:256])
                nc.scalar.copy(outb[:, 256:512], pr[:, 256:512])
                nc.gpsimd.dma_start(out=out_d[img], in_=outb[:])

            for i in range(IMGS + 2):
                if i < IMGS:
                    emit_l3(i)
                if i == 0:
                    # bulk (L2/L1) matrices after image 0's loads so they
                    # don't steal HBM bandwidth from the startup path
                    nc.gpsimd.dma_start(out=matz[:, MAT_SPLIT:],
                                        in_=matz_d[:, MAT_SPLIT:])
                    nc.gpsimd.dma_start(out=matb[:, MAT_SPLIT:],
                                        in_=matb_d[:, MAT_SPLIT:])
                if 1 <= i <= IMGS:
                    emit_l2(i - 1)
                if i >= 2:
                    emit_l1(i - 2)

    split_excess_waits(nc)
    return nc


# ---------------------------------------------------------------------------
# Entry point
# ---------------------------------------------------------------------------
_NC_CACHE = []
_LAST_RESULT = []


def _axon_reset():
    try:
        import ctypes
        lib = ctypes.CDLL('/opt/axon/libaxon_pjrt.so')
        lib.axon_reset.restype = ctypes.c_int64
        lib.axon_reset()
    except Exception:
        pass


def kernel(yl, yh0, yh1, yh2, g0o, g1o, g0a, g0b, g1a, g1b):
    yl = np.asarray(yl, np.float32)
    yh0 = np.asarray(yh0, np.float32)
    yh1 = np.asarray(yh1, np.float32)
    yh2 = np.asarray(yh2, np.float32)
    assert yl.shape == (8, 16, 64, 64)

    matz, matb = build_matrices(g0o, g1o, g0a, g0b, g1a, g1b)
    if not _NC_CACHE:
        _NC_CACHE.append(build_nc())
    nc = _NC_CACHE[0]

    in_maps = []
    for core in range(N_CORES):
        m = prep_core(yl[core], yh0[core], yh1[core], yh2[core])
        m['matz'] = matz
        m['matb'] = matb
        in_maps.append(m)

    try:
        res = run_bass_kernel_spmd(nc, in_maps, list(range(N_CORES)))
    except Exception as e:
        if "UNAVAILABLE" not in str(e) and "unrecoverable" not in str(e):
            raise
        _axon_reset()
        res = run_bass_kernel_spmd(nc, in_maps, list(range(N_CORES)))
    _LAST_RESULT.clear()
    _LAST_RESULT.append(res)
    outs = []
    for i in range(N_CORES):
        o = res.results[i]["out"]                     # [16, 128, 512]
        o = o.reshape(16, 128, 2, 256).transpose(0, 2, 1, 3).reshape(
            16, 256, 256)
        outs.append(o)
    out = np.stack(outs, axis=0)
    return np.ascontiguousarray(out.astype(np.float32))
